# revision 2
# baseline (speedup 1.0000x reference)
"""Bass/Trainium2 kernel for nn_DynamicToepliztMultiheadV2 — v2: 2-level
block-Karatsuba.

Math: out[b,h,t,e] = sum_s w_h[t-s] * x[b,h,s,e], w_h[d] = DPB-MLP(d)[h],
d in [-4095, 4095].  Toeplitz matmul per head; head-parallel across 8 cores
(core c owns head c; [4096,4096] x [4096,512]).

v2: two Karatsuba levels on the 32x32 block-Toeplitz:
  y_top = T0 xt + T- xb ; y_bot = T+ xt + T0 xb
  P_S = T0 (xt+xb); P_A = (T- - T0) xb; P_B = (T+ - T0) xt
  y_top = P_S + P_A ; y_bot = P_S + P_B
recursed once inside each half-product -> 9 leaf products of 8x8 blocks
= 576 matmuls (vs 1024 dense).  Leaf Toeplitz matrices are elementwise
combos of w at shifts {0,+-1024,+-2048,+-3072}: built as vector combos
(DVE on a (128,64) view of wrev; shift 1024 == 16 partitions), then
Hankel-DMA + PE column-flip materializes the 135 leaf blocks.  Matmuls in
bf16 (1 cyc/row + FWL); accumulation fp32 in PSUM; shared leaf outputs
evacuated bf16 and re-added on DVE.
"""
import sys
sys.path.insert(0, "/opt/trn_rl_repo")

import numpy as np
import concourse.bass as bass
import concourse.bacc as bacc
import concourse.mybir as mybir
import concourse.tile as tile
from concourse.ap import AP
from concourse.bass_utils import run_bass_kernel_spmd
from contextlib import ExitStack

FP32 = mybir.dt.float32
FP32R = mybir.dt.float32r
BF16 = mybir.dt.bfloat16
ACT = mybir.ActivationFunctionType

B, H, N, E, PD = 8, 8, 4096, 64, 16
NB = N // 128           # 32 seq blocks
COLS = B * E            # 512
LN_EPS = 1e-5
MROWS = 8192
MCOLS = MROWS // 8

# 9 leaf families as {shift_in_1024_units: coeff} combos of w(t + 1024*s).
FAMS = {
    ("S", "S"): {0: 1},
    ("S", "A"): {-1: 1, 0: -1},
    ("S", "B"): {1: 1, 0: -1},
    ("A", "S"): {-2: 1, 0: -1},
    ("A", "A"): {-3: 1, -2: -1, -1: -1, 0: 1},
    ("A", "B"): {-1: 1, -2: -1, 1: -1, 0: 1},
    ("B", "S"): {2: 1, 0: -1},
    ("B", "A"): {1: 1, -1: -1, 2: -1, 0: 1},
    ("B", "B"): {3: 1, 1: -1, 2: -1, 0: 1},
}
FAM_LIST = list(FAMS.keys())
FAM_IDX = {k: i for i, k in enumerate(FAM_LIST)}

_CACHED_NC = {}


def _build_nc(repeat=1):
    nc = bacc.Bacc("TRN2", target_bir_lowering=False, debug=False)

    xh = nc.declare_dram_parameter("xh", [N, COLS], FP32, isOutput=False)
    tvals = nc.declare_dram_parameter("tvals", [128, MCOLS], FP32R, isOutput=False)
    vecs = nc.declare_dram_parameter("vecs", [10, 128, 1], FP32, isOutput=False)
    bds = nc.declare_dram_parameter("bds", [7, 128, 128], FP32R, isOutput=False)
    jrev = nc.declare_dram_parameter("jrev", [128, 128], FP32, isOutput=False)
    idmp = nc.declare_dram_parameter("idmp", [128, 128], FP32, isOutput=False)
    out = nc.declare_dram_parameter("out", [N, COLS], FP32, isOutput=True)

    wrev = nc.dram_tensor("wrev", [MROWS], FP32R)
    cfs = nc.dram_tensor("cfs", [9 * 2048], BF16)

    with tile.TileContext(nc) as tc:
        with ExitStack() as ctx:
            xpool = ctx.enter_context(tc.tile_pool(name="xpool", bufs=1))
            cpool = ctx.enter_context(tc.tile_pool(name="cpool", bufs=1))
            mpool = ctx.enter_context(tc.tile_pool(name="mpool", bufs=1))
            mqpool = ctx.enter_context(tc.tile_pool(name="mqpool", bufs=3))
            tpool = ctx.enter_context(tc.tile_pool(name="tpool", bufs=1))
            epool = ctx.enter_context(tc.tile_pool(name="epool", bufs=1))
            opool = ctx.enter_context(tc.tile_pool(name="opool", bufs=4))

            # ---- MLP constants
            tv = cpool.tile([128, MCOLS], FP32R, tag="tv")
            nc.sync.dma_start(tv[:], tvals[:])
            vbig = cpool.tile([128, 10], FP32, tag="vbig")
            nc.sync.dma_start(vbig[:], AP(tensor=vecs[:].tensor, offset=0,
                                          ap=[[1, 128], [128, 10]]))
            vtiles = [vbig[:, r:r + 1] for r in range(10)]
            w0v, b0v, g1v, be1v, g2v, be2v, g3v, be3v, b3v, epsv = vtiles
            bdbig = cpool.tile([128, 7 * 128], FP32R, tag="bdbig")
            nc.sync.dma_start(bdbig[:], AP(tensor=bds[:].tensor, offset=0,
                                           ap=[[128, 128], [128 * 128, 7], [1, 128]]))
            btiles = [bdbig[:, r * 128:(r + 1) * 128] for r in range(7)]
            (bd_cent, bd_mean, bd_w1, bd_w2, bd_w3,
             bd_cw1, bd_cw2) = btiles
            jr = cpool.tile([128, 128], BF16, tag="jr")
            nc.gpsimd.dma_start(jr[:], jrev[:])  # fp32 -> bf16 cast DMA
            idm = cpool.tile([128, 128], BF16, tag="idm")
            nc.gpsimd.dma_start(idm[:], idmp[:])

            # ---- load x (bf16 cast): xbig[q, (j, c)] = xh[128j+q, c]
            xbig = xpool.tile([128, NB * COLS], BF16, tag="xbig")
            nc.gpsimd.dma_start(
                xbig[:], AP(tensor=xh[:].tensor, offset=0,
                            ap=[[COLS, 128], [128 * COLS, NB], [1, COLS]]))
            X = [xbig[:, j * COLS:(j + 1) * COLS] for j in range(NB)]

            for rep in range(repeat):
                mlp_psum_scope = tc.tile_pool(name=f"mpsum{rep}", bufs=1, space="PSUM")
                mpsum = mlp_psum_scope.__enter__()

                # ---- MLP (as baseline)
                cur = mpool.tile([128, MCOLS], FP32R, tag="h0")
                nc.scalar.activation(cur[:], tv[:], ACT.Identity, bias=b0v, scale=w0v)
                HALF = MCOLS // 2
                gs = [g1v, g2v, g3v]
                bes = [be1v, be2v, be3v]
                cmats = [bd_cent, bd_cw1, bd_cw2]
                A = cur
                for li in range(3):
                    C = mpsum.tile([128, MCOLS], FP32, tag="c")
                    for hf in range(2):
                        sl = slice(hf * HALF, (hf + 1) * HALF)
                        nc.tensor.matmul(C[:, sl], cmats[li], A[:, sl],
                                         start=True, stop=True)
                    A = mpool.tile([128, MCOLS], FP32R, tag="a")
                    QH = MCOLS // 4
                    for qf in range(4):
                        sl = slice(qf * QH, (qf + 1) * QH)
                        S = mqpool.tile([128, QH], FP32R, tag="s")
                        nc.scalar.activation(S[:], C[:, sl], ACT.Square)
                        V = mpsum.tile([128, QH], FP32, tag=f"v{qf}")
                        nc.tensor.matmul(V[:], bd_mean, S[:],
                                         start=True, stop=True)
                        SD = mqpool.tile([128, QH], FP32, tag="sd")
                        nc.scalar.activation(SD[:], V[:], ACT.Sqrt, bias=epsv)
                        INV = mqpool.tile([128, QH], FP32, tag="inv")
                        nc.vector.reciprocal_approx_fast(INV[:], SD[:])
                        NRM = mqpool.tile([128, QH], FP32, tag="nrm")
                        nc.vector.tensor_mul(NRM[:], C[:, sl], INV[:])
                        nc.scalar.activation(A[:, sl], NRM[:], ACT.Relu,
                                             bias=bes[li], scale=gs[li])
                Hp = mpsum.tile([128, MCOLS], FP32, tag="h")
                for hf in range(2):
                    sl = slice(hf * HALF, (hf + 1) * HALF)
                    nc.tensor.matmul(Hp[:, sl], bd_w3, A[:, sl],
                                     start=True, stop=True)
                cur = mpool.tile([128, MCOLS], FP32R, tag="h0")
                nc.scalar.activation(cur[:], Hp[:], ACT.Identity, bias=b3v)

                # wrev[g*1024 + col] = cur[16g, col]  (w reversed: wrev[r]=w(4095-r))
                src_ap = AP(tensor=cur[:].tensor, offset=0,
                            ap=[[16 * MCOLS, 8], [1, MCOLS]])
                dst_ap = AP(tensor=wrev[:].tensor, offset=0,
                            ap=[[MCOLS, 8], [1, MCOLS]])
                nc.sync.dma_start(dst_ap, src_ap)
                mlp_psum_scope.__exit__(None, None, None)

                # ---- leaf-family weight vectors
                # w(t + 1024*s) at combo index i (= 64*pp + c, t = 1023 - i)
                # equals wrev[3072 + i - 1024*s].  Load all 7 shifts s=-3..3
                # partition-aligned in ONE DMA: wstack[pp, (sigma, c)] =
                # wrev[6144 - 1024*sigma + 64*pp + c], sigma = s + 3.
                wstack = epool.tile([32, 7 * 64], FP32R, tag="wstack")
                nc.sync.dma_start(
                    wstack[:], AP(tensor=wrev[:].tensor, offset=6144,
                                  ap=[[64, 32], [-1024, 7], [1, 64]]))
                cfall = epool.tile([32, 9 * 64], FP32R, tag="cfall")

                def wslice(s):
                    sigma = s + 3
                    return wstack[:, sigma * 64:(sigma + 1) * 64]

                for (key, terms) in FAMS.items():
                    f = FAM_IDX[key]
                    dst = cfall[:, f * 64:(f + 1) * 64]
                    items = sorted(terms.items())
                    if len(items) == 1:
                        nc.vector.tensor_copy(dst, wslice(items[0][0]))
                    else:
                        pos = [s for s, c in items if c > 0]
                        neg = [s for s, c in items if c < 0]
                        # start with pos0 - neg0 (every family has >=1 of each)
                        nc.vector.tensor_sub(dst, wslice(pos[0]), wslice(neg[0]))
                        for s in pos[1:]:
                            nc.vector.tensor_add(dst, dst, wslice(s))
                        for s in neg[1:]:
                            nc.vector.tensor_sub(dst, dst, wslice(s))
                # cfs[f*2048 + i] = cfall[pp, f*64+c]  (fp32 -> bf16 cast)
                nc.gpsimd.dma_start(
                    AP(tensor=cfs[:].tensor, offset=0,
                       ap=[[64, 32], [2048, 9], [1, 64]]),
                    AP(tensor=cfall[:].tensor, offset=0,
                       ap=[[9 * 64, 32], [64, 9], [1, 64]]))

                # ---- x-side Karatsuba combos (DVE), bf16
                xcmb = xpool.tile([128, 40 * COLS], BF16, tag="xcmb")
                xS = [xcmb[:, j * COLS:(j + 1) * COLS] for j in range(16)]
                xS2 = [xcmb[:, (16 + j) * COLS:(17 + j) * COLS] for j in range(8)]
                xb2 = [xcmb[:, (24 + j) * COLS:(25 + j) * COLS] for j in range(8)]
                xt2 = [xcmb[:, (32 + j) * COLS:(33 + j) * COLS] for j in range(8)]
                for j in range(16):
                    nc.vector.tensor_add(xS[j], X[j], X[j + 16])
                for j in range(8):
                    nc.vector.tensor_add(xS2[j], xS[j], xS[j + 8])
                    nc.vector.tensor_add(xb2[j], X[16 + j], X[24 + j])
                    nc.vector.tensor_add(xt2[j], X[j], X[8 + j])
                U = {
                    ("S", "S"): xS2, ("S", "A"): xS[8:16], ("S", "B"): xS[0:8],
                    ("A", "S"): xb2, ("A", "A"): X[24:32], ("A", "B"): X[16:24],
                    ("B", "S"): xt2, ("B", "A"): X[8:16], ("B", "B"): X[0:8],
                }

                # ---- leaf Toeplitz blocks: Hankel load + PE column flip
                # hk[q, (dblk, p)] = cfs[f*2048 + (1792 - 128*dblk) + q + p]
                # pt[i,j] = hk[127-j, i] = f(128*(dblk-7) + j - i); lhsT use.
                # contiguous per-partition Hankel: hkf[q, i] = cfs[f*2048+q+i];
                # slice m (i = 128m..128m+127) is the dblk = 14-m window.
                Tleaf = {}
                with tc.tile_pool(name=f"tpsum{rep}", bufs=4, space="PSUM") as tpsum, \
                     tc.tile_pool(name=f"hkpool{rep}", bufs=4) as hkpool:
                    for key in FAM_LIST:
                        f = FAM_IDX[key]
                        hkf = hkpool.tile([128, 1920], BF16, tag="hkf")
                        nc.sync.dma_start(
                            hkf[:], AP(tensor=cfs[:].tensor, offset=f * 2048,
                                       ap=[[1, 128], [1, 1920]]))
                        for m in range(15):
                            d = 7 - m
                            pt = tpsum.tile([128, 128], FP32, tag="pt")
                            nc.tensor.matmul(pt[:],
                                             hkf[:, m * 128:(m + 1) * 128],
                                             jr[:], start=True, stop=True)
                            tt = tpool.tile([128, 128], BF16, tag=f"t{f}_{d}")
                            if m % 2 == 0:
                                nc.vector.tensor_copy(tt[:], pt[:])
                            else:
                                nc.scalar.activation(tt[:], pt[:], ACT.Copy)
                            Tleaf[(key, d)] = tt

                # ---- main Karatsuba matmuls
                ev = epool.tile([128, 24 * COLS], BF16, tag="ev")
                OSS = [ev[:, i * COLS:(i + 1) * COLS] for i in range(8)]
                OSA = [ev[:, (8 + i) * COLS:(9 + i) * COLS] for i in range(8)]
                OSB = [ev[:, (16 + i) * COLS:(17 + i) * COLS] for i in range(8)]
                OCORE = {"S": OSS, "A": OSA, "B": OSB}
                rsbuf = epool.tile([128, 16 * COLS], BF16, tag="rs")
                RS = [rsbuf[:, i * COLS:(i + 1) * COLS] for i in range(16)]

                ppsum_scope = tc.tile_pool(name=f"ppsum{rep}", bufs=6, space="PSUM")
                ppsum = ppsum_scope.__enter__()

                def leaf_mms(P, key, i, extra=()):
                    u = U[key]
                    for j in range(8):
                        nc.tensor.matmul(P[:], Tleaf[(key, i - j)][:], u[j],
                                         start=(j == 0),
                                         stop=(j == 7 and not extra))
                    for n_, ex in enumerate(extra):
                        nc.tensor.matmul(P[:], idm[:], ex,
                                         start=False,
                                         stop=(n_ == len(extra) - 1))

                # cores o_{beta,S}[i] -> bf16 evac via ACT
                for beta in ("S", "A", "B"):
                    for i in range(8):
                        P = ppsum.tile([128, COLS], FP32, tag="p")
                        leaf_mms(P, (beta, "S"), i)
                        nc.scalar.activation(OCORE[beta][i], P[:], ACT.Copy)

                # r_S[ip] = o_SS[ip%8] + o_S{A|B}[ip]  (P_S, bf16)
                for ip in range(16):
                    i = ip % 8
                    gam = "A" if ip < 8 else "B"
                    P = ppsum.tile([128, COLS], FP32, tag="p")
                    leaf_mms(P, ("S", gam), i, extra=[OSS[i]])
                    nc.scalar.activation(RS[ip], P[:], ACT.Copy)

                # finals: y[16*half + ip] = RS[ip] + o_{beta,S}[i] + psum
                for half, beta in ((0, "A"), (1, "B")):
                    for ip in range(16):
                        i = ip % 8
                        gam = "A" if ip < 8 else "B"
                        P = ppsum.tile([128, COLS], FP32, tag="p")
                        leaf_mms(P, (beta, gam), i, extra=[OCORE[beta][i], RS[ip]])
                        O = opool.tile([128, COLS], FP32, tag="o")
                        nc.scalar.activation(O[:], P[:], ACT.Copy)
                        iblk = half * 16 + ip
                        dst = AP(tensor=out[:].tensor, offset=128 * iblk * COLS,
                                 ap=[[COLS, 128], [1, COLS]])
                        nc.sync.dma_start(dst, O[:])

                ppsum_scope.__exit__(None, None, None)
    nc.compile()
    return nc


def _host_inputs(h, x, W0, b0, g1, be1, W1, b1, g2, be2, W2, b2, g3, be3, W3, b3):
    """Per-core input map for head h."""
    xh = np.ascontiguousarray(
        np.asarray(x)[:, h].transpose(1, 0, 2).reshape(N, COLS)
    ).astype(np.float32, copy=False)

    g = np.arange(8)
    col = np.arange(MCOLS)
    tpos = (4095.0 - (g[:, None] * MCOLS + col[None, :])).astype(np.float32)
    tvals = np.repeat(tpos, PD, axis=0)

    def rep(v):
        return np.tile(np.asarray(v, np.float32).reshape(-1), 8)[:, None]

    b3p = np.zeros(PD, np.float32)
    b3p[0] = b3[h]
    vecs = np.stack([
        rep(W0[0]), rep(b0), rep(g1), rep(be1), rep(g2), rep(be2),
        rep(g3), rep(be3), rep(b3p),
        np.full((128, 1), LN_EPS, np.float32),
    ]).astype(np.float32)

    I16 = np.eye(PD, dtype=np.float32)
    J16 = np.full((PD, PD), 1.0 / PD, np.float32)
    w3c = np.zeros((PD, PD), np.float32)
    w3c[:, 0] = W3[:, h]
    cent16 = I16 - J16
    W1f = np.asarray(W1, np.float32)
    W2f = np.asarray(W2, np.float32)
    I8 = np.eye(8, dtype=np.float32)
    bds = np.stack([
        np.kron(I8, cent16),
        np.kron(I8, J16),
        np.kron(I8, W1f),
        np.kron(I8, W2f),
        np.kron(I8, w3c),
        np.kron(I8, W1f @ cent16),
        np.kron(I8, W2f @ cent16),
    ]).astype(np.float32)

    jrev = np.eye(128, dtype=np.float32)[:, ::-1].copy()
    idmp = np.eye(128, dtype=np.float32)
    return {"xh": xh, "tvals": tvals, "vecs": vecs, "bds": bds, "jrev": jrev,
            "idmp": idmp}


def kernel(x, W0, b0, g1, be1, W1, b1, g2, be2, W2, b2, g3, be3, W3, b3,
           _want_results=False, _trace=False, _repeat=1):
    if _repeat not in _CACHED_NC:
        _CACHED_NC[_repeat] = _build_nc(_repeat)
    nc = _CACHED_NC[_repeat]

    args = (x, W0, b0, g1, be1, W1, b1, g2, be2, W2, b2, g3, be3, W3, b3)
    in_maps = [_host_inputs(h, *args) for h in range(H)]
    res = run_bass_kernel_spmd(nc, in_maps, list(range(H)), trace=_trace)

    outf = np.empty((B, H, N, E), np.float32)
    for h in range(H):
        outf[:, h] = res.results[h]["out"].reshape(N, B, E).transpose(1, 0, 2)
    if _want_results:
        return outf, res
    return outf


# revision 3
# speedup vs baseline: 1.3102x; 1.3102x over previous
"""Bass/Trainium2 kernel for nn_DynamicToepliztMultiheadV2 — v2: 2-level
block-Karatsuba.

Math: out[b,h,t,e] = sum_s w_h[t-s] * x[b,h,s,e], w_h[d] = DPB-MLP(d)[h],
d in [-4095, 4095].  Toeplitz matmul per head; head-parallel across 8 cores
(core c owns head c; [4096,4096] x [4096,512]).

v2: two Karatsuba levels on the 32x32 block-Toeplitz:
  y_top = T0 xt + T- xb ; y_bot = T+ xt + T0 xb
  P_S = T0 (xt+xb); P_A = (T- - T0) xb; P_B = (T+ - T0) xt
  y_top = P_S + P_A ; y_bot = P_S + P_B
recursed once inside each half-product -> 9 leaf products of 8x8 blocks
= 576 matmuls (vs 1024 dense).  Leaf Toeplitz matrices are elementwise
combos of w at shifts {0,+-1024,+-2048,+-3072}: built as vector combos
(DVE on a (128,64) view of wrev; shift 1024 == 16 partitions), then
Hankel-DMA + PE column-flip materializes the 135 leaf blocks.  Matmuls in
bf16 (1 cyc/row + FWL); accumulation fp32 in PSUM; shared leaf outputs
evacuated bf16 and re-added on DVE.
"""
import sys
sys.path.insert(0, "/opt/trn_rl_repo")

import numpy as np
import concourse.bass as bass
import concourse.bacc as bacc
import concourse.mybir as mybir
import concourse.tile as tile
from concourse.ap import AP
from concourse.bass_utils import run_bass_kernel_spmd
from contextlib import ExitStack

FP32 = mybir.dt.float32
FP32R = mybir.dt.float32r
BF16 = mybir.dt.bfloat16
ACT = mybir.ActivationFunctionType

B, H, N, E, PD = 8, 8, 4096, 64, 16
NB = N // 128           # 32 seq blocks
COLS = B * E            # 512
LN_EPS = 1e-5
MROWS = 8192
MCOLS = MROWS // 8

# 9 leaf families as {shift_in_1024_units: coeff} combos of w(t + 1024*s).
FAMS = {
    ("S", "S"): {0: 1},
    ("S", "A"): {-1: 1, 0: -1},
    ("S", "B"): {1: 1, 0: -1},
    ("A", "S"): {-2: 1, 0: -1},
    ("A", "A"): {-3: 1, -2: -1, -1: -1, 0: 1},
    ("A", "B"): {-1: 1, -2: -1, 1: -1, 0: 1},
    ("B", "S"): {2: 1, 0: -1},
    ("B", "A"): {1: 1, -1: -1, 2: -1, 0: 1},
    ("B", "B"): {3: 1, 1: -1, 2: -1, 0: 1},
}
FAM_LIST = list(FAMS.keys())
FAM_IDX = {k: i for i, k in enumerate(FAM_LIST)}

_CACHED_NC = {}


def _build_nc(repeat=1):
    nc = bacc.Bacc("TRN2", target_bir_lowering=False, debug=False)

    xh = nc.declare_dram_parameter("xh", [N, COLS], FP32, isOutput=False)
    tvals = nc.declare_dram_parameter("tvals", [128, MCOLS], FP32R, isOutput=False)
    vecs = nc.declare_dram_parameter("vecs", [10, 128, 1], FP32, isOutput=False)
    bds = nc.declare_dram_parameter("bds", [7, 128, 128], FP32R, isOutput=False)
    jrev = nc.declare_dram_parameter("jrev", [128, 128], FP32, isOutput=False)
    idmp = nc.declare_dram_parameter("idmp", [128, 128], FP32, isOutput=False)
    out = nc.declare_dram_parameter("out", [N, COLS], FP32, isOutput=True)

    wrev = nc.dram_tensor("wrev", [MROWS], FP32R)
    cfs = nc.dram_tensor("cfs", [9 * 2048], BF16)

    with tile.TileContext(nc) as tc:
        with ExitStack() as ctx:
            xpool = ctx.enter_context(tc.tile_pool(name="xpool", bufs=1))
            cpool = ctx.enter_context(tc.tile_pool(name="cpool", bufs=1))
            mpool = ctx.enter_context(tc.tile_pool(name="mpool", bufs=1))
            mqpool = ctx.enter_context(tc.tile_pool(name="mqpool", bufs=4))
            tpool = ctx.enter_context(tc.tile_pool(name="tpool", bufs=1))
            epool = ctx.enter_context(tc.tile_pool(name="epool", bufs=1))
            opool = ctx.enter_context(tc.tile_pool(name="opool", bufs=4))
            mpsum = ctx.enter_context(tc.tile_pool(name="mpsum", bufs=1, space="PSUM"))
            ppsum = ctx.enter_context(tc.tile_pool(name="ppsum", bufs=3, space="PSUM"))

            # ---- MLP constants
            tv = cpool.tile([128, MCOLS], FP32R, tag="tv")
            nc.sync.dma_start(tv[:], tvals[:])
            vbig = cpool.tile([128, 10], FP32, tag="vbig")
            nc.sync.dma_start(vbig[:], AP(tensor=vecs[:].tensor, offset=0,
                                          ap=[[1, 128], [128, 10]]))
            vtiles = [vbig[:, r:r + 1] for r in range(10)]
            w0v, b0v, g1v, be1v, g2v, be2v, g3v, be3v, b3v, epsv = vtiles
            bdbig = cpool.tile([128, 7 * 128], FP32R, tag="bdbig")
            nc.sync.dma_start(bdbig[:], AP(tensor=bds[:].tensor, offset=0,
                                           ap=[[128, 128], [128 * 128, 7], [1, 128]]))
            btiles = [bdbig[:, r * 128:(r + 1) * 128] for r in range(7)]
            (bd_cent, bd_mean, bd_w1, bd_w2, bd_w3,
             bd_cw1, bd_cw2) = btiles
            jr = cpool.tile([128, 128], BF16, tag="jr")
            nc.gpsimd.dma_start(jr[:], jrev[:])  # fp32 -> bf16 cast DMA
            idm = cpool.tile([128, 128], BF16, tag="idm")
            nc.gpsimd.dma_start(idm[:], idmp[:])

            # ---- load x (bf16 cast): xbig[q, (j, c)] = xh[128j+q, c]
            xbig = xpool.tile([128, NB * COLS], BF16, tag="xbig")
            nc.gpsimd.dma_start(
                xbig[:], AP(tensor=xh[:].tensor, offset=0,
                            ap=[[COLS, 128], [128 * COLS, NB], [1, COLS]]))
            X = [xbig[:, j * COLS:(j + 1) * COLS] for j in range(NB)]

            for rep in range(repeat):
                # ---- MLP: wave-order emission, psum = c0,c1,v0,v1 (4 banks)
                cur = mpool.tile([128, MCOLS], FP32R, tag="h0")
                nc.scalar.activation(cur[:], tv[:], ACT.Identity, bias=b0v, scale=w0v)
                HALF = MCOLS // 2
                QH = MCOLS // 4
                gs = [g1v, g2v, g3v]
                bes = [be1v, be2v, be3v]
                cmats = [bd_cent, bd_cw1, bd_cw2]
                A = cur
                for li in range(3):
                    ch = []
                    for hf in range(2):
                        C = mpsum.tile([128, HALF], FP32, tag=f"c{hf}")
                        nc.tensor.matmul(C[:], cmats[li],
                                         A[:, hf * HALF:(hf + 1) * HALF],
                                         start=True, stop=True)
                        ch.append(C)

                    def Cq(q):
                        return ch[q // 2][:, (q % 2) * QH:(q % 2 + 1) * QH]

                    Ss = []
                    for q in range(4):
                        S = mqpool.tile([128, QH], FP32R, tag="s")
                        nc.scalar.activation(S[:], Cq(q), ACT.Square)
                        Ss.append(S)
                    Vs = []
                    for q in range(4):
                        V = mpsum.tile([128, QH], FP32, tag=f"v{q % 2}")
                        nc.tensor.matmul(V[:], bd_mean, Ss[q][:],
                                         start=True, stop=True)
                        Vs.append(V)
                    SDs = []
                    for q in range(4):
                        SD = mqpool.tile([128, QH], FP32, tag="sd")
                        nc.scalar.activation(SD[:], Vs[q][:], ACT.Sqrt, bias=epsv)
                        SDs.append(SD)
                    INVs = []
                    for q in range(4):
                        INV = mqpool.tile([128, QH], FP32, tag="inv")
                        nc.vector.reciprocal_approx_fast(INV[:], SDs[q][:])
                        INVs.append(INV)
                    NRMs = []
                    for q in range(4):
                        NRM = mqpool.tile([128, QH], FP32, tag="nrm")
                        nc.vector.tensor_mul(NRM[:], Cq(q), INVs[q][:])
                        NRMs.append(NRM)
                    Anew = mpool.tile([128, MCOLS], FP32R, tag="a")
                    for q in range(4):
                        nc.scalar.activation(Anew[:, q * QH:(q + 1) * QH],
                                             NRMs[q][:], ACT.Relu,
                                             bias=bes[li], scale=gs[li])
                    A = Anew
                hh = []
                for hf in range(2):
                    Hp = mpsum.tile([128, HALF], FP32, tag=f"c{hf}")
                    nc.tensor.matmul(Hp[:], bd_w3,
                                     A[:, hf * HALF:(hf + 1) * HALF],
                                     start=True, stop=True)
                    hh.append(Hp)
                cur = mpool.tile([128, MCOLS], FP32R, tag="h0")
                for hf in range(2):
                    nc.scalar.activation(cur[:, hf * HALF:(hf + 1) * HALF],
                                         hh[hf][:], ACT.Identity, bias=b3v)

                # wrev[g*1024 + col] = cur[16g, col]  (w reversed: wrev[r]=w(4095-r))
                src_ap = AP(tensor=cur[:].tensor, offset=0,
                            ap=[[16 * MCOLS, 8], [1, MCOLS]])
                dst_ap = AP(tensor=wrev[:].tensor, offset=0,
                            ap=[[MCOLS, 8], [1, MCOLS]])
                nc.sync.dma_start(dst_ap, src_ap)

                # ---- leaf-family weight vectors
                # w(t + 1024*s) at combo index i (= 64*pp + c, t = 1023 - i)
                # equals wrev[3072 + i - 1024*s].  Load all 7 shifts s=-3..3
                # partition-aligned in ONE DMA: wstack[pp, (sigma, c)] =
                # wrev[6144 - 1024*sigma + 64*pp + c], sigma = s + 3.
                wstack = epool.tile([32, 7 * 64], FP32R, tag="wstack")
                nc.sync.dma_start(
                    wstack[:], AP(tensor=wrev[:].tensor, offset=6144,
                                  ap=[[64, 32], [-1024, 7], [1, 64]]))
                cfall = epool.tile([32, 9 * 64], FP32R, tag="cfall")

                def wslice(s):
                    sigma = s + 3
                    return wstack[:, sigma * 64:(sigma + 1) * 64]

                for (key, terms) in FAMS.items():
                    f = FAM_IDX[key]
                    dst = cfall[:, f * 64:(f + 1) * 64]
                    items = sorted(terms.items())
                    if len(items) == 1:
                        nc.vector.tensor_copy(dst, wslice(items[0][0]))
                    else:
                        pos = [s for s, c in items if c > 0]
                        neg = [s for s, c in items if c < 0]
                        # start with pos0 - neg0 (every family has >=1 of each)
                        nc.vector.tensor_sub(dst, wslice(pos[0]), wslice(neg[0]))
                        for s in pos[1:]:
                            nc.vector.tensor_add(dst, dst, wslice(s))
                        for s in neg[1:]:
                            nc.vector.tensor_sub(dst, dst, wslice(s))
                # cfs[f*2048 + i] = cfall[pp, f*64+c]  (fp32 -> bf16 cast)
                nc.gpsimd.dma_start(
                    AP(tensor=cfs[:].tensor, offset=0,
                       ap=[[64, 32], [2048, 9], [1, 64]]),
                    AP(tensor=cfall[:].tensor, offset=0,
                       ap=[[9 * 64, 32], [64, 9], [1, 64]]))

                # ---- x-side Karatsuba combos (DVE), bf16
                xcmb = xpool.tile([128, 40 * COLS], BF16, tag="xcmb")
                xS = [xcmb[:, j * COLS:(j + 1) * COLS] for j in range(16)]
                xS2 = [xcmb[:, (16 + j) * COLS:(17 + j) * COLS] for j in range(8)]
                xb2 = [xcmb[:, (24 + j) * COLS:(25 + j) * COLS] for j in range(8)]
                xt2 = [xcmb[:, (32 + j) * COLS:(33 + j) * COLS] for j in range(8)]
                for j in range(16):
                    nc.vector.tensor_add(xS[j], X[j], X[j + 16])
                for j in range(8):
                    nc.vector.tensor_add(xS2[j], xS[j], xS[j + 8])
                    nc.vector.tensor_add(xb2[j], X[16 + j], X[24 + j])
                    nc.vector.tensor_add(xt2[j], X[j], X[8 + j])
                U = {
                    ("S", "S"): xS2, ("S", "A"): xS[8:16], ("S", "B"): xS[0:8],
                    ("A", "S"): xb2, ("A", "A"): X[24:32], ("A", "B"): X[16:24],
                    ("B", "S"): xt2, ("B", "A"): X[8:16], ("B", "B"): X[0:8],
                }

                # ---- leaf Toeplitz blocks: Hankel load + PE column flip
                # hk[q, (dblk, p)] = cfs[f*2048 + (1792 - 128*dblk) + q + p]
                # pt[i,j] = hk[127-j, i] = f(128*(dblk-7) + j - i); lhsT use.
                # contiguous per-partition Hankel: hkf[q, i] = cfs[f*2048+q+i];
                # slice m (i = 128m..128m+127) is the dblk = 14-m window.
                Tleaf = {}
                with tc.tile_pool(name=f"hkpool{rep}", bufs=4) as hkpool:
                    for key in FAM_LIST:
                        f = FAM_IDX[key]
                        hkf = hkpool.tile([128, 1920], BF16, tag="hkf")
                        nc.sync.dma_start(
                            hkf[:], AP(tensor=cfs[:].tensor, offset=f * 2048,
                                       ap=[[1, 128], [1, 1920]]))
                        for m in range(15):
                            d = 7 - m
                            pt = ppsum.tile([128, COLS], FP32, tag="p")
                            nc.tensor.matmul(pt[:, 0:128],
                                             hkf[:, m * 128:(m + 1) * 128],
                                             jr[:], start=True, stop=True)
                            tt = tpool.tile([128, 128], BF16, tag=f"t{f}_{d}")
                            if m % 2 == 0:
                                nc.vector.tensor_copy(tt[:], pt[:, 0:128])
                            else:
                                nc.scalar.activation(tt[:], pt[:, 0:128], ACT.Copy)
                            Tleaf[(key, d)] = tt

                # ---- main Karatsuba matmuls
                ev = epool.tile([128, 24 * COLS], BF16, tag="ev")
                OSS = [ev[:, i * COLS:(i + 1) * COLS] for i in range(8)]
                OSA = [ev[:, (8 + i) * COLS:(9 + i) * COLS] for i in range(8)]
                OSB = [ev[:, (16 + i) * COLS:(17 + i) * COLS] for i in range(8)]
                OCORE = {"S": OSS, "A": OSA, "B": OSB}
                rsbuf = epool.tile([128, 16 * COLS], BF16, tag="rs")
                RS = [rsbuf[:, i * COLS:(i + 1) * COLS] for i in range(16)]

                def leaf_mms(P, key, i, extra=()):
                    u = U[key]
                    for j in range(8):
                        nc.tensor.matmul(P[:], Tleaf[(key, i - j)][:], u[j],
                                         start=(j == 0),
                                         stop=(j == 7 and not extra))
                    for n_, ex in enumerate(extra):
                        nc.tensor.matmul(P[:], idm[:], ex,
                                         start=False,
                                         stop=(n_ == len(extra) - 1))

                # cores o_{beta,S}[i] -> bf16 evac via ACT
                for beta in ("S", "A", "B"):
                    for i in range(8):
                        P = ppsum.tile([128, COLS], FP32, tag="p")
                        leaf_mms(P, (beta, "S"), i)
                        nc.scalar.activation(OCORE[beta][i], P[:], ACT.Copy)

                # r_S[ip] = o_SS[ip%8] + o_S{A|B}[ip]  (P_S, bf16)
                for ip in range(16):
                    i = ip % 8
                    gam = "A" if ip < 8 else "B"
                    P = ppsum.tile([128, COLS], FP32, tag="p")
                    leaf_mms(P, ("S", gam), i, extra=[OSS[i]])
                    nc.scalar.activation(RS[ip], P[:], ACT.Copy)

                # finals: y[16*half + ip] = RS[ip] + o_{beta,S}[i] + psum
                for half, beta in ((0, "A"), (1, "B")):
                    for ip in range(16):
                        i = ip % 8
                        gam = "A" if ip < 8 else "B"
                        P = ppsum.tile([128, COLS], FP32, tag="p")
                        leaf_mms(P, (beta, gam), i, extra=[OCORE[beta][i], RS[ip]])
                        O = opool.tile([128, COLS], FP32, tag="o")
                        nc.scalar.activation(O[:], P[:], ACT.Copy)
                        iblk = half * 16 + ip
                        dst = AP(tensor=out[:].tensor, offset=128 * iblk * COLS,
                                 ap=[[COLS, 128], [1, COLS]])
                        nc.sync.dma_start(dst, O[:])

    nc.compile()
    return nc


def _host_inputs(h, x, W0, b0, g1, be1, W1, b1, g2, be2, W2, b2, g3, be3, W3, b3):
    """Per-core input map for head h."""
    xh = np.ascontiguousarray(
        np.asarray(x)[:, h].transpose(1, 0, 2).reshape(N, COLS)
    ).astype(np.float32, copy=False)

    g = np.arange(8)
    col = np.arange(MCOLS)
    tpos = (4095.0 - (g[:, None] * MCOLS + col[None, :])).astype(np.float32)
    tvals = np.repeat(tpos, PD, axis=0)

    def rep(v):
        return np.tile(np.asarray(v, np.float32).reshape(-1), 8)[:, None]

    b3p = np.zeros(PD, np.float32)
    b3p[0] = b3[h]
    vecs = np.stack([
        rep(W0[0]), rep(b0), rep(g1), rep(be1), rep(g2), rep(be2),
        rep(g3), rep(be3), rep(b3p),
        np.full((128, 1), LN_EPS, np.float32),
    ]).astype(np.float32)

    I16 = np.eye(PD, dtype=np.float32)
    J16 = np.full((PD, PD), 1.0 / PD, np.float32)
    w3c = np.zeros((PD, PD), np.float32)
    w3c[:, 0] = W3[:, h]
    cent16 = I16 - J16
    W1f = np.asarray(W1, np.float32)
    W2f = np.asarray(W2, np.float32)
    I8 = np.eye(8, dtype=np.float32)
    bds = np.stack([
        np.kron(I8, cent16),
        np.kron(I8, J16),
        np.kron(I8, W1f),
        np.kron(I8, W2f),
        np.kron(I8, w3c),
        np.kron(I8, W1f @ cent16),
        np.kron(I8, W2f @ cent16),
    ]).astype(np.float32)

    jrev = np.eye(128, dtype=np.float32)[:, ::-1].copy()
    idmp = np.eye(128, dtype=np.float32)
    return {"xh": xh, "tvals": tvals, "vecs": vecs, "bds": bds, "jrev": jrev,
            "idmp": idmp}


def kernel(x, W0, b0, g1, be1, W1, b1, g2, be2, W2, b2, g3, be3, W3, b3,
           _want_results=False, _trace=False, _repeat=1):
    if _repeat not in _CACHED_NC:
        _CACHED_NC[_repeat] = _build_nc(_repeat)
    nc = _CACHED_NC[_repeat]

    args = (x, W0, b0, g1, be1, W1, b1, g2, be2, W2, b2, g3, be3, W3, b3)
    in_maps = [_host_inputs(h, *args) for h in range(H)]
    res = run_bass_kernel_spmd(nc, in_maps, list(range(H)), trace=_trace)

    outf = np.empty((B, H, N, E), np.float32)
    for h in range(H):
        outf[:, h] = res.results[h]["out"].reshape(N, B, E).transpose(1, 0, 2)
    if _want_results:
        return outf, res
    return outf
